# revision 1
# baseline (speedup 1.0000x reference)
"""Trainium2 Bass kernel for windowed-style attention with relative position bias.

Shapes (hardcoded): x [4, 2048, 512], H=8 heads, HD=64, rel table [4098, 8].

Sharding: 8 cores = 4 batches x 2 query-halves. Each core computes the full
attention + projection for its 1024 query rows of its batch (keys span all
2048 tokens), so outputs are disjoint row slices -- no collectives.

Device dataflow (per core, identical SPMD program):
  - qT/kT = W @ xT (PE, fp16 in / fp32 accum; q pre-scaled by HD^-0.5)
  - V computed in natural [token, d] layout with a ones column appended per
    head (gives the softmax denominator for free in the AV matmul)
  - scores are computed transposed (S^T: keys on partitions, queries free),
    softmax reduction over keys happens inside the PE via the ones column;
    no max-subtraction pass is needed (scores ~ N(0,1), exp is safe)
  - E = exp(S^T) (ACT) * exp(bias)^T (DVE, fp16) -- the relative-position
    bias is applied multiplicatively with a host-precomputed exp'ed table
  - O^T accumulated per head in PSUM, normalized by the PE-broadcast
    reciprocal of the denominator row, projection back to [token, C] (PE)

Heads are processed in pairs: the two K=64 score matmuls land on disjoint
PE row-groups (partitions 0:64 / 64:128) so the hardware runs them
concurrently, and the pair shares one [128, 1024] PSUM tile so exp (ACT)
and the bias multiply (DVE) run 1024-wide, halving instruction overheads.
The program is software-pipelined: V and the next pair's q/k projections are
interleaved into the attention loops so PE-heavy projection work overlaps
the ACT-bound softmax stretch.
"""

import sys

sys.path.insert(0, "/opt/trn_rl_repo")

import numpy as np

import concourse.mybir as mybir
import concourse.tile as tile
from concourse import bacc
from concourse.bass import ds, ts
from concourse.bass_utils import run_bass_kernel_spmd

B, N, C, H, HD = 4, 2048, 512, 8, 64
NQ = N // 2
NCORES = 8
SCALE = HD ** -0.5
F32 = mybir.dt.float32
F16 = mybir.dt.float16
EXP = mybir.ActivationFunctionType.Exp
COPY = mybir.ActivationFunctionType.Copy


def build_kernel(reps=1):
    nc = bacc.Bacc("TRN2", target_bir_lowering=False, debug=False, num_devices=NCORES)

    xT = nc.dram_tensor("xT", [C, N], F16, kind="ExternalInput").ap()
    wqT = nc.dram_tensor("wqT", [C, C], F16, kind="ExternalInput").ap()
    wkT = nc.dram_tensor("wkT", [C, C], F16, kind="ExternalInput").ap()
    wvT = nc.dram_tensor("wvT", [C, C], F16, kind="ExternalInput").ap()
    wp8 = nc.dram_tensor("wp8", [64, 8, C], F16, kind="ExternalInput").ap()
    bbr = nc.dram_tensor("bbr", [128, C], F32, kind="ExternalInput").ap()
    # [head-pair g, query-block nb, key%128, key//128, head-parity, query]
    eb = nc.dram_tensor("eb", [4, 2, 128, 16, 2, 512], F16, kind="ExternalInput").ap()
    out = nc.dram_tensor("out", [NQ, C], F32, kind="ExternalOutput").ap()

    with tile.TileContext(nc) as tc:
        with (
            tc.tile_pool(name="const", bufs=1) as Kc,
            tc.tile_pool(name="ebp", bufs=2) as Keb,
            tc.tile_pool(name="ep", bufs=4) as Kep,
            tc.tile_pool(name="rp", bufs=3) as Krp,
            tc.tile_pool(name="osb", bufs=2) as Kosb,
            tc.tile_pool(name="outp", bufs=3) as Kout,
            tc.tile_pool(name="ps", bufs=2, space="PSUM") as Kps,
            tc.tile_pool(name="pso", bufs=2, space="PSUM") as Kpso,
            tc.tile_pool(name="psa", bufs=2, space="PSUM") as Kpsa,
        ):
            xT_s = Kc.tile([128, 4, N], F16, name="xT_s")
            wq_s = Kc.tile([128, 4, C], F16, name="wq_s")
            wk_s = Kc.tile([128, 4, C], F16, name="wk_s")
            wv_s = Kc.tile([128, 4, C], F16, name="wv_s")
            wp_s = Kc.tile([64, 8, C], F16, name="wp_s")
            ones16 = Kc.tile([128, 64], F16, name="ones16")
            qT_s = Kc.tile([128, 4, NQ], F16, name="qT_s")
            kT_s = Kc.tile([128, 4, N], F16, name="kT_s")
            va_s = Kc.tile([128, 16, H, HD + 1], F16, name="va_s")
            ot_s = Kc.tile([64, 8, NQ], F16, name="ot_s")
            bb_s = Kc.tile([128, C], F32, name="bb_s")

            r128 = lambda ap: ap.rearrange("(po pi) t -> pi po t", pi=128)
            xT_r, wq_r = r128(xT), r128(wqT)

            # blocks of phase-B work in processing order; eb prefetched one
            # block ahead
            blocks = [(nb, g) for nb in range(2) for g in range(4)]
            eb_tiles = {}
            rep_body = []  # deferred emission so the body can repeat (timing)

            def emit_eb_load(i, nsplit=4):
                nb, g = blocks[i]
                t = eb_tiles.get(i)
                if t is None:
                    t = Keb.tile([128, 16, 2, 512], F16, tag="eb")
                    eb_tiles[i] = t
                step = 16 // nsplit
                for q in range(nsplit):  # split so the first chunks land early
                    nc.sync.dma_start(
                        t[:, ds(step * q, step)], eb[g, nb, :, ds(step * q, step)]
                    )

            def emit_body():
                # loads, ordered so the first q/k matmuls and eb block start early
                nc.sync.dma_start(bb_s, bbr)
                nc.sync.dma_start(wq_s[:, 0, :], wq_r[:, 0, :])
                nc.sync.dma_start(wk_s, r128(wkT))
                for c in range(4):
                    nc.sync.dma_start(xT_s[:, c, :], xT_r[:, c, :])
                nc.sync.dma_start(wv_s, r128(wvT))
                for c in range(1, 4):
                    nc.sync.dma_start(wq_s[:, c, :], wq_r[:, c, :])
                emit_eb_load(0)
                nc.sync.dma_start(wp_s, wp8)
                nc.vector.memset(ones16, 1.0)
                nc.vector.memset(va_s, 1.0)  # ones col survives; V cols overwritten


                def emit_qT(ot, cb):
                    ps = Kpsa.tile([128, 512], F32, tag="psa")
                    for c in range(4):
                        nc.tensor.matmul(
                            ps,
                            lhsT=wq_s[:, c, ts(ot, 128)],
                            rhs=xT_s[:, c, ts(cb, 512)],
                            start=(c == 0),
                            stop=(c == 3),
                        )
                    nc.vector.tensor_scalar_mul(qT_s[:, ot, ts(cb, 512)], ps, float(SCALE))

                def emit_kT(ot, cb):
                    ps = Kpsa.tile([128, 512], F32, tag="psa")
                    for c in range(4):
                        nc.tensor.matmul(
                            ps,
                            lhsT=wk_s[:, c, ts(ot, 128)],
                            rhs=xT_s[:, c, ts(cb, 512)],
                            start=(c == 0),
                            stop=(c == 3),
                        )
                    nc.vector.tensor_copy(kT_s[:, ot, ts(cb, 512)], ps)

                def emit_V(tt):
                    ps = Kpsa.tile([128, 512], F32, tag="psa")
                    for c in range(4):
                        nc.tensor.matmul(
                            ps,
                            lhsT=xT_s[:, c, ts(tt, 128)],
                            rhs=wv_s[:, c, :],
                            start=(c == 0),
                            stop=(c == 3),
                        )
                    nc.vector.tensor_copy(
                        va_s[:, tt, :, 0:HD], ps.rearrange("p (h d) -> p h d", h=H)
                    )

                def emit_B_iter(nb, g, mt, eb_t, o_ps):
                    s_ps = Kps.tile([128, 1024], F32, tag="ps")
                    for par in range(2):  # head 2g+par on PE rows par*64
                        nc.tensor.matmul(
                            s_ps[:, ts(par, 512)],
                            lhsT=kT_s[par * 64 : par * 64 + 64, g, ts(mt, 128)],
                            rhs=qT_s[par * 64 : par * 64 + 64, g, ts(nb, 512)],
                            start=True,
                            stop=True,
                        )
                    e_t = Kep.tile([128, 1024], F16, tag="e")
                    nc.scalar.activation(e_t, s_ps, EXP)
                    mul_eng = nc.vector
                    mul_eng.tensor_mul(
                        e_t, e_t, eb_t[:, mt, :, :].rearrange("p a b -> p (a b)")
                    )
                    for par in range(2):
                        nc.tensor.matmul(
                            o_ps[par][0 : HD + 1, :],
                            lhsT=va_s[:, mt, 2 * g + par, :],
                            rhs=e_t[:, ts(par, 512)],
                            start=(mt == 0),
                            stop=(mt == 15),
                        )

                def emit_o_copy(o_ps):
                    # free the PSUM accumulators at block end; normalize later
                    o_sb = Kosb.tile([128, 2, 512], F16, tag="osb")
                    for par in range(2):
                        nc.scalar.activation(
                            o_sb[0 : HD + 1, par, :], o_ps[par][0 : HD + 1, :], COPY
                        )
                    return o_sb

                def emit_norm_direct(nb, g, o_ps):
                    for par in range(2):
                        h = 2 * g + par
                        r_t = Krp.tile([128, 512], F16, tag="r")
                        with nc.allow_low_precision("softmax normalization in fp16"):
                            nc.vector.reciprocal(r_t[64:65, :], o_ps[par][64:65, :])
                        rb_ps = Kpsa.tile([128, 512], F32, tag="psa")
                        nc.tensor.matmul(
                            rb_ps[0:64, :],
                            lhsT=ones16[64:65, 0:64],
                            rhs=r_t[64:65, :],
                            start=True,
                            stop=True,
                        )
                        rb_s = Krp.tile([128, 512], F16, tag="rb_s")
                        nc.vector.tensor_copy(rb_s[0:64, :], rb_ps[0:64, :])
                        nc.vector.tensor_mul(
                            ot_s[:, h, ts(nb, 512)], o_ps[par][0:64, :], rb_s[0:64, :]
                        )

                def emit_norm(nb, g, o_sb):
                    for par in range(2):
                        h = 2 * g + par
                        r_t = Krp.tile([128, 512], F16, tag="r")
                        with nc.allow_low_precision("softmax normalization in fp16"):
                            nc.vector.reciprocal(r_t[64:65, :], o_sb[64:65, par, :])
                        rb_ps = Kpsa.tile([128, 512], F32, tag="psa")
                        nc.tensor.matmul(
                            rb_ps[0:64, :],
                            lhsT=ones16[64:65, 0:64],
                            rhs=r_t[64:65, :],
                            start=True,
                            stop=True,
                        )
                        rb_s = Krp.tile([128, 512], F16, tag="rb_s")
                        nc.vector.tensor_copy(rb_s[0:64, :], rb_ps[0:64, :])
                        nc.vector.tensor_mul(
                            ot_s[:, h, ts(nb, 512)], o_sb[0:64, par, :], rb_s[0:64, :]
                        )

                def emit_proj(nb, ns, pool=None):
                    p_ps = (pool or Kpsa).tile(
                        [128, 512], F32, tag="psa" if pool is None else "o"
                    )
                    for c8 in range(8):
                        nc.tensor.matmul(
                            p_ps,
                            lhsT=ot_s[:, c8, ds(nb * 512 + ns * 128, 128)],
                            rhs=wp_s[:, c8, :],
                            start=(c8 == 0),
                            stop=(c8 == 7),
                        )
                    o_t = Kout.tile([128, 512], F32, tag="out")
                    nc.vector.tensor_add(o_t, p_ps, bb_s)
                    nc.sync.dma_start(out[ds(nb * 512 + ns * 128, 128), :], o_t)

                # ---- pipelined schedule ----
                emit_qT(0, 0)
                emit_qT(0, 1)
                for cb in range(4):
                    emit_kT(0, cb)

                pending_norm = None
                for i, (nb, g) in enumerate(blocks):
                    if i + 1 < len(blocks):
                        emit_eb_load(i + 1)
                    # projection-type PE work to interleave into this block
                    filler = []
                    if nb == 0 and g < 3:
                        filler += [lambda ot=g + 1, cb=cb: emit_qT(ot, cb) for cb in range(2)]
                        filler += [lambda ot=g + 1, cb=cb: emit_kT(ot, cb) for cb in range(4)]
                    if nb == 1 and g == 0:
                        filler += [lambda ns=ns: emit_proj(0, ns) for ns in range(4)]
                    o_lo = Kpso.tile([128, 512], F32, tag="o")
                    o_hi = Kpso.tile([128, 512], F32, tag="o")
                    o_ps = [o_lo, o_hi]
                    eb_t = eb_tiles.pop(i)
                    for mt in range(16):
                        if nb == 0 and g == 0:
                            emit_V(mt)
                        emit_B_iter(nb, g, mt, eb_t, o_ps)
                        if mt == 0 and pending_norm is not None:
                            pending_norm()
                            pending_norm = None
                        if False:  # fillers run at block end; in-loop injection hurt
                            filler.pop(0)()
                    for f in filler:
                        f()
                    if i == len(blocks) - 1:
                        emit_norm_direct(nb, g, o_ps)
                        pending_norm = None
                    else:
                        o_sb = emit_o_copy(o_ps)
                        pending_norm = lambda nb=nb, g=g, o_sb=o_sb: emit_norm(
                            nb, g, o_sb
                        )
                if pending_norm is not None:
                    pending_norm()
                for ns in range(4):
                    emit_proj(1, ns, pool=Kpso if ns % 2 else None)


            for _ in range(reps):
                emit_body()

    nc.compile()
    return nc


_NC = None


def _get_nc():
    global _NC
    if _NC is None:
        _NC = build_kernel()
    return _NC


def _prepare_in_maps(x, w_qkv, rel_bias_table, w_proj, b_proj, mask, rel_idx):
    xf = np.asarray(x, dtype=np.float32)
    wf = np.asarray(w_qkv, dtype=np.float32)
    wq = np.ascontiguousarray(wf[0:C].T.astype(np.float16))
    wk = np.ascontiguousarray(wf[C : 2 * C].T.astype(np.float16))
    wv = np.ascontiguousarray(wf[2 * C : 3 * C].T.astype(np.float16))
    wpT = np.asarray(w_proj, dtype=np.float32).T  # [ci, co]
    wp8_a = np.ascontiguousarray(
        wpT.reshape(8, 64, C).transpose(1, 0, 2).astype(np.float16)
    )
    bb = np.ascontiguousarray(
        np.broadcast_to(np.asarray(b_proj, dtype=np.float32).reshape(1, C), (128, C))
    )

    # exp'ed relative-position bias, transposed to [head, key, query]
    t_exp = np.exp(np.asarray(rel_bias_table, dtype=np.float32)).astype(np.float16)
    idx = np.asarray(rel_idx)
    lut = t_exp[idx]  # [n, m, H] fp16
    ebt = np.ascontiguousarray(lut.transpose(2, 1, 0))  # [H, m, n]

    mask_a = np.asarray(mask)
    all_true = bool(mask_a.all())

    def eb_half(ebt_b, half):
        sl = ebt_b[:, :, half * NQ : (half + 1) * NQ]  # [H, 2048, 1024]
        if half == 1:
            # keys follow the core's permuted token order (own half first)
            sl = np.concatenate([sl[:, NQ:, :], sl[:, :NQ, :]], axis=1)
        # axes: [g, parity, mt, p, nb, n] -> [g, nb, p, mt, parity, n]
        a6 = sl.reshape(4, 2, 16, 128, 2, 512)
        return np.ascontiguousarray(a6.transpose(0, 4, 3, 2, 1, 5))

    eb_shared = None
    if all_true:
        eb_shared = [eb_half(ebt, 0), eb_half(ebt, 1)]

    x16 = xf.astype(np.float16)
    in_maps = []
    for core in range(NCORES):
        b, half = divmod(core, 2)
        if all_true:
            eb_c = eb_shared[half]
        else:
            ebt_b = ebt * mask_a[b].astype(np.float16)[None, :, None]
            eb_c = eb_half(ebt_b, half)
        xb = x16[b]
        if half == 1:
            xb = np.concatenate([xb[NQ:], xb[:NQ]], axis=0)
        in_maps.append(
            {
                "xT": np.ascontiguousarray(xb.T),
                "wqT": wq,
                "wkT": wk,
                "wvT": wv,
                "wp8": wp8_a,
                "bbr": bb,
                "eb": eb_c,
            }
        )
    return in_maps


def _run(inputs, trace=False):
    nc = _get_nc()
    in_maps = _prepare_in_maps(**inputs)
    res = run_bass_kernel_spmd(nc, in_maps, core_ids=list(range(NCORES)), trace=trace)
    outp = np.empty((B, N, C), dtype=np.float32)
    for core in range(NCORES):
        b, half = divmod(core, 2)
        outp[b, half * NQ : (half + 1) * NQ] = res.results[core]["out"]
    return outp, res


def kernel(**inputs) -> np.ndarray:
    outp, _ = _run(inputs, trace=False)
    return outp



# revision 7
# speedup vs baseline: 1.0063x; 1.0063x over previous
"""Trainium2 Bass kernel for windowed-style attention with relative position bias.

Shapes (hardcoded): x [4, 2048, 512], H=8 heads, HD=64, rel table [4098, 8].

Sharding: 8 cores = 4 batches x 2 query-halves. Each core computes the full
attention + projection for its 1024 query rows of its batch (keys span all
2048 tokens), so outputs are disjoint row slices -- no collectives.

Device dataflow (per core, identical SPMD program):
  - qT/kT = W @ xT (PE, fp16 in / fp32 accum; SCALE folded into wq on host)
  - V computed in natural [token, d] layout with a ones column appended per
    head (gives the softmax denominator for free in the AV matmul)
  - scores are computed transposed (S^T: keys on partitions, queries free),
    softmax reduction over keys happens inside the PE via the ones column;
    no max-subtraction pass is needed (scores ~ N(0,1), exp is safe)
  - E = exp(S^T) (ACT) * exp(bias)^T (DVE/Pool, fp16) -- the relative-position
    bias is applied multiplicatively with a host-precomputed exp'ed table
  - O^T accumulated per head in PSUM, normalized by a gpsimd-broadcast
    reciprocal of the denominator row, projection back to [token, C] (PE)

Engine assignment targets an ACT(exp)-bound steady state: all PSUM->SBUF
copies and some bias multiplies run on the Pool/GpSimd engine, the rest of
the bias multiplies and the normalization on DVE, exp on ACT, matmuls on PE
(score head-pairs on disjoint 64-row groups run concurrently).
"""

import sys

sys.path.insert(0, "/opt/trn_rl_repo")

import numpy as np

import concourse.mybir as mybir
import concourse.tile as tile
from concourse import bacc
from concourse.bass import ds, ts
from concourse.bass_utils import run_bass_kernel_spmd

B, N, C, H, HD = 4, 2048, 512, 8, 64
NQ = N // 2
NCORES = 8
SCALE = HD ** -0.5
F32 = mybir.dt.float32
F16 = mybir.dt.float16
EXP = mybir.ActivationFunctionType.Exp
COPY = mybir.ActivationFunctionType.Copy

# which of the 128 (block, mt) iterations run the bias multiply on Pool
# (0 disables Pool multiplies)
POOL_MULT_EVERY = 0


def build_kernel(reps=1):
    nc = bacc.Bacc("TRN2", target_bir_lowering=False, debug=False, num_devices=NCORES)

    xT = nc.dram_tensor("xT", [C, N], F16, kind="ExternalInput").ap()
    wqT = nc.dram_tensor("wqT", [C, C], F16, kind="ExternalInput").ap()
    wkT = nc.dram_tensor("wkT", [C, C], F16, kind="ExternalInput").ap()
    wvT = nc.dram_tensor("wvT", [C, C], F16, kind="ExternalInput").ap()
    wp8 = nc.dram_tensor("wp8", [64, 8, C], F16, kind="ExternalInput").ap()
    bbr = nc.dram_tensor("bbr", [128, C], F32, kind="ExternalInput").ap()
    # [head-pair g, query-block nb, key%128, key//128, head-parity, query]
    eb = nc.dram_tensor("eb", [4, 2, 128, 16, 2, 512], F16, kind="ExternalInput").ap()
    out = nc.dram_tensor("out", [NQ, C], F32, kind="ExternalOutput").ap()

    with tile.TileContext(nc) as tc:
        with (
            tc.tile_pool(name="const", bufs=1) as Kc,
            tc.tile_pool(name="ebp", bufs=2) as Keb,
            tc.tile_pool(name="ep", bufs=4) as Kep,
            tc.tile_pool(name="rp", bufs=3) as Krp,
            tc.tile_pool(name="osb", bufs=2) as Kosb,
            tc.tile_pool(name="outp", bufs=3) as Kout,
            tc.tile_pool(name="ps", bufs=2, space="PSUM") as Kps,
            tc.tile_pool(name="pso", bufs=2, space="PSUM") as Kpso,
            tc.tile_pool(name="psa", bufs=2, space="PSUM") as Kpsa,
        ):
            xT_s = Kc.tile([128, 4, N], F16, name="xT_s")
            wq_s = Kc.tile([128, 4, C], F16, name="wq_s")
            wk_s = Kc.tile([128, 4, C], F16, name="wk_s")
            wv_s = Kc.tile([128, 4, C], F16, name="wv_s")
            wp_s = Kc.tile([64, 8, C], F16, name="wp_s")
            qT_s = Kc.tile([128, 4, NQ], F16, name="qT_s")
            kT_s = Kc.tile([128, 4, N], F16, name="kT_s")
            va_s = Kc.tile([128, 16, H, HD + 1], F16, name="va_s")
            ot_s = Kc.tile([64, 8, NQ], F16, name="ot_s")
            bb_s = Kc.tile([128, C], F32, name="bb_s")

            r128 = lambda ap: ap.rearrange("(po pi) t -> pi po t", pi=128)
            xT_r, wq_r = r128(xT), r128(wqT)

            # blocks of phase-B work in processing order; eb prefetched one
            # block ahead
            blocks = [(nb, g) for nb in range(2) for g in range(4)]
            eb_tiles = {}

            def emit_eb_load(i, nsplit=4):
                nb, g = blocks[i]
                t = eb_tiles.get(i)
                if t is None:
                    t = Keb.tile([128, 16, 2, 512], F16, tag="eb")
                    eb_tiles[i] = t
                step = 16 // nsplit
                for q in range(nsplit):  # split so the first chunks land early
                    nc.sync.dma_start(
                        t[:, ds(step * q, step)], eb[g, nb, :, ds(step * q, step)]
                    )

            def emit_body():
                # loads, ordered so the first q/k matmuls and eb block start early
                nc.sync.dma_start(bb_s, bbr)
                nc.sync.dma_start(wq_s[:, 0, :], wq_r[:, 0, :])
                nc.sync.dma_start(wk_s, r128(wkT))
                for c in range(4):
                    nc.sync.dma_start(xT_s[:, c, :], xT_r[:, c, :])
                nc.sync.dma_start(wv_s, r128(wvT))
                for c in range(1, 4):
                    nc.sync.dma_start(wq_s[:, c, :], wq_r[:, c, :])
                emit_eb_load(0)
                nc.sync.dma_start(wp_s, wp8)
                # ones columns for the softmax denominator; V body columns are
                # overwritten by the V copies
                nc.vector.memset(va_s[:, :, :, HD : HD + 1], 1.0)

                def emit_qT(ot, cb):
                    ps = Kpsa.tile([128, 512], F32, tag="psa")
                    for c in range(4):
                        nc.tensor.matmul(
                            ps,
                            lhsT=wq_s[:, c, ts(ot, 128)],
                            rhs=xT_s[:, c, ts(cb, 512)],
                            start=(c == 0),
                            stop=(c == 3),
                        )
                    nc.vector.tensor_copy(qT_s[:, ot, ts(cb, 512)], ps)

                def emit_kT(ot, cb):
                    ps = Kpsa.tile([128, 512], F32, tag="psa")
                    for c in range(4):
                        nc.tensor.matmul(
                            ps,
                            lhsT=wk_s[:, c, ts(ot, 128)],
                            rhs=xT_s[:, c, ts(cb, 512)],
                            start=(c == 0),
                            stop=(c == 3),
                        )
                    nc.vector.tensor_copy(kT_s[:, ot, ts(cb, 512)], ps)

                def emit_V(tt):
                    ps = Kpsa.tile([128, 512], F32, tag="psa")
                    for c in range(4):
                        nc.tensor.matmul(
                            ps,
                            lhsT=xT_s[:, c, ts(tt, 128)],
                            rhs=wv_s[:, c, :],
                            start=(c == 0),
                            stop=(c == 3),
                        )
                    nc.vector.tensor_copy(
                        va_s[:, tt, :, 0:HD], ps.rearrange("p (h d) -> p h d", h=H)
                    )

                def emit_B_iter(nb, g, mt, eb_t, o_ps, on_pool):
                    s_ps = Kps.tile([128, 1024], F32, tag="ps")
                    for par in range(2):  # head 2g+par on PE rows par*64
                        nc.tensor.matmul(
                            s_ps[:, ts(par, 512)],
                            lhsT=kT_s[par * 64 : par * 64 + 64, g, ts(mt, 128)],
                            rhs=qT_s[par * 64 : par * 64 + 64, g, ts(nb, 512)],
                            start=True,
                            stop=True,
                        )
                    e_t = Kep.tile([128, 1024], F16, tag="e")
                    nc.scalar.activation(e_t, s_ps, EXP)
                    mul_eng = nc.gpsimd if on_pool else nc.vector
                    mul_eng.tensor_mul(
                        e_t, e_t, eb_t[:, mt, :, :].rearrange("p a b -> p (a b)")
                    )
                    for par in range(2):
                        nc.tensor.matmul(
                            o_ps[par][0 : HD + 1, :],
                            lhsT=va_s[:, mt, 2 * g + par, :],
                            rhs=e_t[:, ts(par, 512)],
                            start=(mt == 0),
                            stop=(mt == 15),
                        )

                def emit_o_copy(o_ps):
                    # free the PSUM accumulators at block end; normalize later
                    o_sb = Kosb.tile([128, 2, 512], F16, tag="osb")
                    for par in range(2):
                        nc.vector.tensor_copy(
                            o_sb[0 : HD + 1, par, :], o_ps[par][0 : HD + 1, :]
                        )
                    return o_sb

                def emit_norm_common(nb, g, par, src_recip, src_mult):
                    # src_recip: [1, 512] denominator row; src_mult: [64, 512]
                    h = 2 * g + par
                    r_t = Krp.tile([128, 512], F16, tag="r")
                    with nc.allow_low_precision("softmax normalization in fp16"):
                        nc.vector.reciprocal(r_t[0:1, :], src_recip)
                    rb_s = Krp.tile([128, 512], F16, tag="rb_s")
                    nc.gpsimd.partition_broadcast(rb_s[0:64, :], r_t[0:1, :])
                    nc.vector.tensor_mul(
                        ot_s[:, h, ts(nb, 512)], src_mult, rb_s[0:64, :]
                    )

                def emit_norm_direct(nb, g, o_ps):
                    for par in range(2):
                        emit_norm_common(
                            nb, g, par, o_ps[par][64:65, :], o_ps[par][0:64, :]
                        )

                def emit_norm(nb, g, o_sb):
                    for par in range(2):
                        emit_norm_common(
                            nb, g, par, o_sb[64:65, par, :], o_sb[0:64, par, :]
                        )

                def emit_proj(nb, ns, pool=None):
                    p_ps = (pool or Kpsa).tile(
                        [128, 512], F32, tag="psa" if pool is None else "o"
                    )
                    for c8 in range(8):
                        nc.tensor.matmul(
                            p_ps,
                            lhsT=ot_s[:, c8, ds(nb * 512 + ns * 128, 128)],
                            rhs=wp_s[:, c8, :],
                            start=(c8 == 0),
                            stop=(c8 == 7),
                        )
                    o_t = Kout.tile([128, 512], F32, tag="out")
                    nc.vector.tensor_add(o_t, p_ps, bb_s)
                    nc.sync.dma_start(out[ds(nb * 512 + ns * 128, 128), :], o_t)

                # ---- pipelined schedule ----
                emit_qT(0, 0)
                emit_qT(0, 1)
                for cb in range(4):
                    emit_kT(0, cb)

                pending_norm = None
                it = 0
                for i, (nb, g) in enumerate(blocks):
                    if i + 1 < len(blocks):
                        emit_eb_load(i + 1)
                    # projection-type PE work to interleave into this block
                    filler = []
                    if nb == 0 and g < 3:
                        filler += [lambda ot=g + 1, cb=cb: emit_qT(ot, cb) for cb in range(2)]
                        filler += [lambda ot=g + 1, cb=cb: emit_kT(ot, cb) for cb in range(4)]
                    if nb == 1 and g == 0:
                        filler += [lambda ns=ns: emit_proj(0, ns) for ns in range(4)]
                    o_lo = Kpso.tile([128, 512], F32, tag="o")
                    o_hi = Kpso.tile([128, 512], F32, tag="o")
                    o_ps = [o_lo, o_hi]
                    eb_t = eb_tiles.pop(i)
                    for mt in range(16):
                        if nb == 0 and g == 0:
                            emit_V(mt)
                        on_pool = POOL_MULT_EVERY > 0 and (
                            it % POOL_MULT_EVERY == POOL_MULT_EVERY // 2
                        )
                        emit_B_iter(nb, g, mt, eb_t, o_ps, on_pool)
                        it += 1
                        if mt == 0 and pending_norm is not None:
                            pending_norm()
                            pending_norm = None
                    for f in filler:
                        f()
                    if i == len(blocks) - 1:
                        emit_norm_direct(nb, g, o_ps)
                        pending_norm = None
                    else:
                        o_sb = emit_o_copy(o_ps)
                        pending_norm = lambda nb=nb, g=g, o_sb=o_sb: emit_norm(
                            nb, g, o_sb
                        )
                if pending_norm is not None:
                    pending_norm()
                for ns in range(4):
                    emit_proj(1, ns, pool=Kpso if ns % 2 else None)


            for _ in range(reps):
                emit_body()

    nc.compile()
    return nc


_NC = None


def _get_nc():
    global _NC
    if _NC is None:
        _NC = build_kernel()
    return _NC


def _prepare_in_maps(x, w_qkv, rel_bias_table, w_proj, b_proj, mask, rel_idx):
    xf = np.asarray(x, dtype=np.float32)
    wf = np.asarray(w_qkv, dtype=np.float32)
    wq = np.ascontiguousarray((wf[0:C] * SCALE).T.astype(np.float16))
    wk = np.ascontiguousarray(wf[C : 2 * C].T.astype(np.float16))
    wv = np.ascontiguousarray(wf[2 * C : 3 * C].T.astype(np.float16))
    wpT = np.asarray(w_proj, dtype=np.float32).T  # [ci, co]
    wp8_a = np.ascontiguousarray(
        wpT.reshape(8, 64, C).transpose(1, 0, 2).astype(np.float16)
    )
    bb = np.ascontiguousarray(
        np.broadcast_to(np.asarray(b_proj, dtype=np.float32).reshape(1, C), (128, C))
    )

    # exp'ed relative-position bias, transposed to [head, key, query]
    t_exp = np.exp(np.asarray(rel_bias_table, dtype=np.float32)).astype(np.float16)
    idx = np.asarray(rel_idx)
    lut = t_exp[idx]  # [n, m, H] fp16
    ebt = np.ascontiguousarray(lut.transpose(2, 1, 0))  # [H, m, n]

    mask_a = np.asarray(mask)
    all_true = bool(mask_a.all())

    def eb_half(ebt_b, half):
        sl = ebt_b[:, :, half * NQ : (half + 1) * NQ]  # [H, 2048, 1024]
        if half == 1:
            # keys follow the core's permuted token order (own half first)
            sl = np.concatenate([sl[:, NQ:, :], sl[:, :NQ, :]], axis=1)
        # axes: [g, parity, mt, p, nb, n] -> [g, nb, p, mt, parity, n]
        a6 = sl.reshape(4, 2, 16, 128, 2, 512)
        return np.ascontiguousarray(a6.transpose(0, 4, 3, 2, 1, 5))

    eb_shared = None
    if all_true:
        eb_shared = [eb_half(ebt, 0), eb_half(ebt, 1)]

    x16 = xf.astype(np.float16)
    in_maps = []
    for core in range(NCORES):
        b, half = divmod(core, 2)
        if all_true:
            eb_c = eb_shared[half]
        else:
            ebt_b = ebt * mask_a[b].astype(np.float16)[None, :, None]
            eb_c = eb_half(ebt_b, half)
        xb = x16[b]
        if half == 1:
            xb = np.concatenate([xb[NQ:], xb[:NQ]], axis=0)
        in_maps.append(
            {
                "xT": np.ascontiguousarray(xb.T),
                "wqT": wq,
                "wkT": wk,
                "wvT": wv,
                "wp8": wp8_a,
                "bbr": bb,
                "eb": eb_c,
            }
        )
    return in_maps


def _run(inputs, trace=False):
    nc = _get_nc()
    in_maps = _prepare_in_maps(**inputs)
    res = run_bass_kernel_spmd(nc, in_maps, core_ids=list(range(NCORES)), trace=trace)
    outp = np.empty((B, N, C), dtype=np.float32)
    for core in range(NCORES):
        b, half = divmod(core, 2)
        outp[b, half * NQ : (half + 1) * NQ] = res.results[core]["out"]
    return outp, res


def kernel(**inputs) -> np.ndarray:
    outp, _ = _run(inputs, trace=False)
    return outp


# revision 10
# speedup vs baseline: 4.1620x; 4.1358x over previous
"""Trainium2 Bass kernel for windowed-style attention with relative position bias.

Shapes (hardcoded): x [4, 2048, 512], H=8 heads, HD=64, rel table [4098, 8].

Sharding: 8 cores = 4 batches x 2 query-halves. Each core computes the full
attention + projection for its 1024 query rows of its batch (keys span all
2048 tokens), so outputs are disjoint row slices -- no collectives.

Device dataflow (per core, identical SPMD program):
  - qT/kT = W @ xT (PE, fp16 in / fp32 accum; SCALE folded into wq on host)
  - V computed in natural [token, d] layout with a ones column appended per
    head (gives the softmax denominator for free in the AV matmul)
  - scores are computed transposed (S^T: keys on partitions, queries free),
    softmax reduction over keys happens inside the PE via the ones column;
    no max-subtraction pass is needed (scores ~ N(0,1), exp is safe)
  - E = exp(S^T) (ACT) * exp(bias)^T (DVE/Pool, fp16) -- the relative-position
    bias is applied multiplicatively with a host-precomputed exp'ed table
  - O^T accumulated per head in PSUM, normalized by a gpsimd-broadcast
    reciprocal of the denominator row, projection back to [token, C] (PE)

Engine assignment targets an ACT(exp)-bound steady state: all PSUM->SBUF
copies and some bias multiplies run on the Pool/GpSimd engine, the rest of
the bias multiplies and the normalization on DVE, exp on ACT, matmuls on PE
(score head-pairs on disjoint 64-row groups run concurrently).
"""

import sys

sys.path.insert(0, "/opt/trn_rl_repo")

import numpy as np

import concourse.mybir as mybir
import concourse.tile as tile
from concourse import bacc
from concourse.bass import ds, ts
from concourse.bass_utils import run_bass_kernel_spmd

B, N, C, H, HD = 4, 2048, 512, 8, 64
NQ = N // 2
NCORES = 8
SCALE = HD ** -0.5
F32 = mybir.dt.float32
F16 = mybir.dt.float16
EXP = mybir.ActivationFunctionType.Exp
COPY = mybir.ActivationFunctionType.Copy

# which of the 128 (block, mt) iterations run the bias multiply on Pool
# (0 disables Pool multiplies)
POOL_MULT_EVERY = 0


def build_kernel(reps=1):
    nc = bacc.Bacc("TRN2", target_bir_lowering=False, debug=False, num_devices=NCORES)

    xT = nc.dram_tensor("xT", [C, N], F16, kind="ExternalInput").ap()
    wqT = nc.dram_tensor("wqT", [C, C], F16, kind="ExternalInput").ap()
    wkT = nc.dram_tensor("wkT", [C, C], F16, kind="ExternalInput").ap()
    wvT = nc.dram_tensor("wvT", [C, C], F16, kind="ExternalInput").ap()
    wp8 = nc.dram_tensor("wp8", [64, 8, C], F16, kind="ExternalInput").ap()
    bbr = nc.dram_tensor("bbr", [128, C], F32, kind="ExternalInput").ap()
    # [head-pair g, query-block nb, key%128, key//128, head-parity, query]
    eb = nc.dram_tensor("eb", [4, 2, 128, 16, 2, 512], F16, kind="ExternalInput").ap()
    out = nc.dram_tensor("out", [NQ, C], F32, kind="ExternalOutput").ap()

    with tile.TileContext(nc) as tc:
        with (
            tc.tile_pool(name="const", bufs=1) as Kc,
            tc.tile_pool(name="ebp", bufs=2) as Keb,
            tc.tile_pool(name="ep", bufs=4) as Kep,
            tc.tile_pool(name="rp", bufs=3) as Krp,
            tc.tile_pool(name="osb", bufs=2) as Kosb,
            tc.tile_pool(name="outp", bufs=3) as Kout,
            tc.tile_pool(name="ps", bufs=2, space="PSUM") as Kps,
            tc.tile_pool(name="pso", bufs=2, space="PSUM") as Kpso,
            tc.tile_pool(name="psa", bufs=2, space="PSUM") as Kpsa,
        ):
            xT_s = Kc.tile([128, 4, N], F16, name="xT_s")
            wq_s = Kc.tile([128, 4, C], F16, name="wq_s")
            wk_s = Kc.tile([128, 4, C], F16, name="wk_s")
            wv_s = Kc.tile([128, 4, C], F16, name="wv_s")
            wp_s = Kc.tile([64, 8, C], F16, name="wp_s")
            qT_s = Kc.tile([128, 4, NQ], F16, name="qT_s")
            kT_s = Kc.tile([128, 4, N], F16, name="kT_s")
            # V columns 0:64, softmax-denominator ones column at 64, padding
            # up to 96 so the AV matmul's stationary operand is a multiple of
            # 32 columns (M=65 accumulation chains hit a PE slow path)
            va_s = Kc.tile([128, 16, H, 96], F16, name="va_s")
            ot_s = Kc.tile([64, 8, NQ], F16, name="ot_s")
            bb_s = Kc.tile([128, C], F32, name="bb_s")

            r128 = lambda ap: ap.rearrange("(po pi) t -> pi po t", pi=128)
            xT_r, wq_r = r128(xT), r128(wqT)

            # blocks of phase-B work in processing order; eb prefetched one
            # block ahead
            blocks = [(nb, g) for nb in range(2) for g in range(4)]
            eb_tiles = {}

            def emit_eb_load(i, nsplit=4):
                nb, g = blocks[i]
                t = eb_tiles.get(i)
                if t is None:
                    t = Keb.tile([128, 16, 2, 512], F16, tag="eb")
                    eb_tiles[i] = t
                step = 16 // nsplit
                for q in range(nsplit):  # split so the first chunks land early
                    nc.sync.dma_start(
                        t[:, ds(step * q, step)], eb[g, nb, :, ds(step * q, step)]
                    )

            def emit_body():
                # loads, ordered so the first q/k matmuls and eb block start early
                nc.sync.dma_start(bb_s, bbr)
                nc.sync.dma_start(wq_s[:, 0, :], wq_r[:, 0, :])
                nc.sync.dma_start(wk_s, r128(wkT))
                for c in range(4):
                    nc.sync.dma_start(xT_s[:, c, :], xT_r[:, c, :])
                nc.sync.dma_start(wv_s, r128(wvT))
                for c in range(1, 4):
                    nc.sync.dma_start(wq_s[:, c, :], wq_r[:, c, :])
                emit_eb_load(0)
                nc.sync.dma_start(wp_s, wp8)
                # ones columns for the softmax denominator (col 64; cols 65:96
                # are pad whose values are never read); V body columns are
                # overwritten by the V copies
                nc.vector.memset(va_s[:, :, :, HD:96], 1.0)

                def emit_qT(ot, cb):
                    ps = Kpsa.tile([128, 512], F32, tag="psa")
                    for c in range(4):
                        nc.tensor.matmul(
                            ps,
                            lhsT=wq_s[:, c, ts(ot, 128)],
                            rhs=xT_s[:, c, ts(cb, 512)],
                            start=(c == 0),
                            stop=(c == 3),
                        )
                    nc.vector.tensor_copy(qT_s[:, ot, ts(cb, 512)], ps)

                def emit_kT(ot, cb):
                    ps = Kpsa.tile([128, 512], F32, tag="psa")
                    for c in range(4):
                        nc.tensor.matmul(
                            ps,
                            lhsT=wk_s[:, c, ts(ot, 128)],
                            rhs=xT_s[:, c, ts(cb, 512)],
                            start=(c == 0),
                            stop=(c == 3),
                        )
                    nc.vector.tensor_copy(kT_s[:, ot, ts(cb, 512)], ps)

                def emit_V(tt):
                    ps = Kpsa.tile([128, 512], F32, tag="psa")
                    for c in range(4):
                        nc.tensor.matmul(
                            ps,
                            lhsT=xT_s[:, c, ts(tt, 128)],
                            rhs=wv_s[:, c, :],
                            start=(c == 0),
                            stop=(c == 3),
                        )
                    nc.vector.tensor_copy(
                        va_s[:, tt, :, 0:HD], ps.rearrange("p (h d) -> p h d", h=H)
                    )

                def emit_B_iter(nb, g, mt, eb_t, o_ps, on_pool):
                    s_ps = Kps.tile([128, 1024], F32, tag="ps")
                    for par in range(2):  # head 2g+par on PE rows par*64
                        nc.tensor.matmul(
                            s_ps[:, ts(par, 512)],
                            lhsT=kT_s[par * 64 : par * 64 + 64, g, ts(mt, 128)],
                            rhs=qT_s[par * 64 : par * 64 + 64, g, ts(nb, 512)],
                            start=True,
                            stop=True,
                        )
                    e_t = Kep.tile([128, 1024], F16, tag="e")
                    nc.scalar.activation(e_t, s_ps, EXP)
                    mul_eng = nc.gpsimd if on_pool else nc.vector
                    mul_eng.tensor_mul(
                        e_t, e_t, eb_t[:, mt, :, :].rearrange("p a b -> p (a b)")
                    )
                    for par in range(2):
                        nc.tensor.matmul(
                            o_ps[par][0:96, :],
                            lhsT=va_s[:, mt, 2 * g + par, :],
                            rhs=e_t[:, ts(par, 512)],
                            start=(mt == 0),
                            stop=(mt == 15),
                        )

                def emit_o_copy(o_ps):
                    # free the PSUM accumulators at block end; normalize later
                    o_sb = Kosb.tile([128, 2, 512], F16, tag="osb")
                    for par in range(2):
                        nc.vector.tensor_copy(
                            o_sb[0 : HD + 1, par, :], o_ps[par][0 : HD + 1, :]
                        )
                    return o_sb

                def emit_norm_common(nb, g, par, src_recip, src_mult):
                    # src_recip: [1, 512] denominator row; src_mult: [64, 512]
                    h = 2 * g + par
                    r_t = Krp.tile([128, 512], F16, tag="r")
                    with nc.allow_low_precision("softmax normalization in fp16"):
                        nc.vector.reciprocal(r_t[0:1, :], src_recip)
                    rb_s = Krp.tile([128, 512], F16, tag="rb_s")
                    nc.gpsimd.partition_broadcast(rb_s[0:64, :], r_t[0:1, :])
                    nc.vector.tensor_mul(
                        ot_s[:, h, ts(nb, 512)], src_mult, rb_s[0:64, :]
                    )

                def emit_norm_direct(nb, g, o_ps):
                    for par in range(2):
                        emit_norm_common(
                            nb, g, par, o_ps[par][64:65, :], o_ps[par][0:64, :]
                        )

                def emit_norm(nb, g, o_sb):
                    for par in range(2):
                        emit_norm_common(
                            nb, g, par, o_sb[64:65, par, :], o_sb[0:64, par, :]
                        )

                def emit_proj(nb, ns, pool=None):
                    p_ps = (pool or Kpsa).tile(
                        [128, 512], F32, tag="psa" if pool is None else "o"
                    )
                    for c8 in range(8):
                        nc.tensor.matmul(
                            p_ps,
                            lhsT=ot_s[:, c8, ds(nb * 512 + ns * 128, 128)],
                            rhs=wp_s[:, c8, :],
                            start=(c8 == 0),
                            stop=(c8 == 7),
                        )
                    o_t = Kout.tile([128, 512], F32, tag="out")
                    nc.vector.tensor_add(o_t, p_ps, bb_s)
                    nc.sync.dma_start(out[ds(nb * 512 + ns * 128, 128), :], o_t)

                # ---- pipelined schedule ----
                emit_qT(0, 0)
                emit_qT(0, 1)
                for cb in range(4):
                    emit_kT(0, cb)

                pending_norm = None
                it = 0
                for i, (nb, g) in enumerate(blocks):
                    if i + 1 < len(blocks):
                        emit_eb_load(i + 1)
                    # projection-type PE work to interleave into this block
                    filler = []
                    if nb == 0 and g < 3:
                        filler += [lambda ot=g + 1, cb=cb: emit_qT(ot, cb) for cb in range(2)]
                        filler += [lambda ot=g + 1, cb=cb: emit_kT(ot, cb) for cb in range(4)]
                    if nb == 1 and g == 0:
                        filler += [lambda ns=ns: emit_proj(0, ns) for ns in range(4)]
                    o_lo = Kpso.tile([128, 512], F32, tag="o")
                    o_hi = Kpso.tile([128, 512], F32, tag="o")
                    o_ps = [o_lo, o_hi]
                    eb_t = eb_tiles.pop(i)
                    for mt in range(16):
                        if nb == 0 and g == 0:
                            emit_V(mt)
                        on_pool = POOL_MULT_EVERY > 0 and (
                            it % POOL_MULT_EVERY == POOL_MULT_EVERY // 2
                        )
                        emit_B_iter(nb, g, mt, eb_t, o_ps, on_pool)
                        it += 1
                        if mt == 0 and pending_norm is not None:
                            pending_norm()
                            pending_norm = None
                    for f in filler:
                        f()
                    if i == len(blocks) - 1:
                        emit_norm_direct(nb, g, o_ps)
                        pending_norm = None
                    else:
                        o_sb = emit_o_copy(o_ps)
                        pending_norm = lambda nb=nb, g=g, o_sb=o_sb: emit_norm(
                            nb, g, o_sb
                        )
                if pending_norm is not None:
                    pending_norm()
                for ns in range(4):
                    emit_proj(1, ns, pool=Kpso if ns % 2 else None)


            for _ in range(reps):
                emit_body()

    nc.compile()
    return nc


_NC = None


def _get_nc():
    global _NC
    if _NC is None:
        _NC = build_kernel()
    return _NC


def _prepare_in_maps(x, w_qkv, rel_bias_table, w_proj, b_proj, mask, rel_idx):
    xf = np.asarray(x, dtype=np.float32)
    wf = np.asarray(w_qkv, dtype=np.float32)
    wq = np.ascontiguousarray((wf[0:C] * SCALE).T.astype(np.float16))
    wk = np.ascontiguousarray(wf[C : 2 * C].T.astype(np.float16))
    wv = np.ascontiguousarray(wf[2 * C : 3 * C].T.astype(np.float16))
    wpT = np.asarray(w_proj, dtype=np.float32).T  # [ci, co]
    wp8_a = np.ascontiguousarray(
        wpT.reshape(8, 64, C).transpose(1, 0, 2).astype(np.float16)
    )
    bb = np.ascontiguousarray(
        np.broadcast_to(np.asarray(b_proj, dtype=np.float32).reshape(1, C), (128, C))
    )

    # exp'ed relative-position bias, transposed to [head, key, query]
    t_exp = np.exp(np.asarray(rel_bias_table, dtype=np.float32)).astype(np.float16)
    idx = np.asarray(rel_idx)
    lut = t_exp[idx]  # [n, m, H] fp16
    ebt = np.ascontiguousarray(lut.transpose(2, 1, 0))  # [H, m, n]

    mask_a = np.asarray(mask)
    all_true = bool(mask_a.all())

    def eb_half(ebt_b, half):
        sl = ebt_b[:, :, half * NQ : (half + 1) * NQ]  # [H, 2048, 1024]
        if half == 1:
            # keys follow the core's permuted token order (own half first)
            sl = np.concatenate([sl[:, NQ:, :], sl[:, :NQ, :]], axis=1)
        # axes: [g, parity, mt, p, nb, n] -> [g, nb, p, mt, parity, n]
        a6 = sl.reshape(4, 2, 16, 128, 2, 512)
        return np.ascontiguousarray(a6.transpose(0, 4, 3, 2, 1, 5))

    eb_shared = None
    if all_true:
        eb_shared = [eb_half(ebt, 0), eb_half(ebt, 1)]

    x16 = xf.astype(np.float16)
    in_maps = []
    for core in range(NCORES):
        b, half = divmod(core, 2)
        if all_true:
            eb_c = eb_shared[half]
        else:
            ebt_b = ebt * mask_a[b].astype(np.float16)[None, :, None]
            eb_c = eb_half(ebt_b, half)
        xb = x16[b]
        if half == 1:
            xb = np.concatenate([xb[NQ:], xb[:NQ]], axis=0)
        in_maps.append(
            {
                "xT": np.ascontiguousarray(xb.T),
                "wqT": wq,
                "wkT": wk,
                "wvT": wv,
                "wp8": wp8_a,
                "bbr": bb,
                "eb": eb_c,
            }
        )
    return in_maps


def _run(inputs, trace=False):
    nc = _get_nc()
    in_maps = _prepare_in_maps(**inputs)
    res = run_bass_kernel_spmd(nc, in_maps, core_ids=list(range(NCORES)), trace=trace)
    outp = np.empty((B, N, C), dtype=np.float32)
    for core in range(NCORES):
        b, half = divmod(core, 2)
        outp[b, half * NQ : (half + 1) * NQ] = res.results[core]["out"]
    return outp, res


def kernel(**inputs) -> np.ndarray:
    outp, _ = _run(inputs, trace=False)
    return outp
